# revision 19
# baseline (speedup 1.0000x reference)
"""Trainium2 Bass kernel for nn_CADenseMul.

Math (see reference):
    chi  = sigmoid(context @ W + Bc)          # [B, R]
    s    = S * chi                            # [B, R]
    out  = ((inputs @ U) * s) @ V.T + bias    # [B, UNITS]

Strategy:
  - Data-parallel over batch B across 8 cores (B=4096 -> 512 rows/core).
  - Host-side prep (not device time): per-core transposed activation shards
    packed into SBUF-layout blobs ([128, cols] contiguous per partition ->
    line-rate DMA); fold S into U (U_s = U * S); ship V pre-transposed;
    cast streams to bf16.
  - Device (transposed-activation layout, batch as the free dim):
        h.T    = W.T @ ctx.T          (PSUM; sigmoid+Bc on ACT)
        proj.T = U_s.T @ x.T          (per b-slice, pipelined with x loads)
        psT    = proj.T * chi.T       (DVE, cast bf16)
        out    = psT.T @ V.T          (psT stationary, natural-layout out)
  - PE warm-up: dummy matmuls at start so HAM un-throttles before real work.
  - Output stored bf16 (halves store traffic); host concats, adds bias fp32.
"""

import os
import numpy as np
import ml_dtypes

import concourse.bass as bass
import concourse.tile as tile
from concourse import bacc, mybir
from concourse.bass_utils import run_bass_kernel_spmd

N_CORES = 8
B, D_IN, D_CTX, UNITS, R = 4096, 2048, 512, 2048, 256
BS = B // N_CORES        # 512 batch rows per core
KT_X = D_IN // 128       # 16
KT_C = D_CTX // 128      # 4
RT = R // 128            # 2
NBT = BS // 128          # 4 output batch tiles

ACT_DTYPE = os.environ.get("CAD_DTYPE", "bf16")    # bf16 | f32r
OUT_BF16 = os.environ.get("CAD_OUT", "bf16") == "bf16"
NH = int(os.environ.get("CAD_NH", "2"))            # batch slices (2 or 4)
XCH = int(os.environ.get("CAD_XCH", "4"))          # k-chunks per x slice DMA
N_WARM = int(os.environ.get("CAD_WARM", "32"))     # warm-up matmuls
BH = BS // NH

_COMPILED = {}


def _build(key):
    act_dtype, out_bf16, nh, xch, n_warm = key
    dt_act = mybir.dt.bfloat16 if act_dtype == "bf16" else mybir.dt.float32r
    dt_f32 = mybir.dt.float32
    dt_out = mybir.dt.bfloat16 if out_bf16 else dt_f32
    bh = BS // nh
    bt_per_h = NBT // nh

    nc = bacc.Bacc("TRN2", target_bir_lowering=False, debug=False,
                   num_devices=N_CORES)

    # packed blobs: [128, cols] per-partition-contiguous
    wc = nc.dram_tensor("wc", [128, KT_C * R + KT_C * BS], dt_act,
                        kind="ExternalInput").ap()          # W | ctxT
    ub = nc.dram_tensor("ub", [128, KT_X * R], dt_act,
                        kind="ExternalInput").ap()          # U_s
    xh = [nc.dram_tensor(f"xh{j}", [128, KT_X * bh], dt_act,
                         kind="ExternalInput").ap() for j in range(nh)]
    vb = nc.dram_tensor("vb", [128, RT * UNITS], dt_act,
                        kind="ExternalInput").ap()          # V.T
    Bc2 = nc.dram_tensor("Bc2", [128, RT], dt_f32, kind="ExternalInput").ap()
    out = nc.dram_tensor("out", [BS, UNITS], dt_out, kind="ExternalOutput").ap()
    dummy_out = nc.dram_tensor("dummy_out", [128, 24], dt_f32,
                               kind="ExternalOutput").ap()

    with tile.TileContext(nc) as tc:
        with (
            tc.tile_pool(name="consts", bufs=1) as consts,
            tc.tile_pool(name="osb", bufs=2) as osb,
            tc.tile_pool(name="ps_h", bufs=RT, space="PSUM") as ps_h,
            tc.tile_pool(name="ps_p", bufs=2, space="PSUM") as ps_p,
            tc.tile_pool(name="ps_o", bufs=4, space="PSUM") as ps_o,
        ):
            # ---- PE warm-up: garbage matmuls, no data deps ----
            warm_sb = consts.tile([128, 128], dt_act, tag="warm")
            nc.gpsimd.memset(warm_sb[:], 0.0)
            warm_ps = ps_p.tile([128, 128], dt_f32, tag="pps")
            for _ in range(n_warm):
                nc.tensor.matmul(warm_ps[:], warm_sb[:], warm_sb[:],
                                 start=True, stop=True)
            # keep it alive through DCE: route result to a real output
            warm_sink = consts.tile([128, 24], dt_f32, tag="warm_sink")
            nc.vector.tensor_copy(warm_sink[:, :8], warm_ps[:, :8])
            # preload the ACT "Copy" function table while ACT is idle so the
            # final-stage copies don't pay the 1.3us table load mid-kernel
            # (reads warm_sb, NOT warm_ps -- must not wait on the matmuls)
            nc.scalar.activation(warm_sink[:, 8:16], warm_sb[:, :8],
                                 mybir.ActivationFunctionType.Copy)

            # ---- loads: <=8 in flight (HWDGE sem lanes), split so arrival
            # order matches consumption: ub+x0 / wc early, vb mid, x1 last ----
            kpc = KT_X // xch  # k-tiles per x chunk (xch=2 -> halves)
            xh_sb = []
            for j in range(nh):
                xt = consts.tile([128, KT_X * bh], dt_act, tag=f"xh{j}")
                xh_sb.append(xt)

            def load_x_chunk(eng, j, q):
                c0, c1 = q * kpc * bh, (q + 1) * kpc * bh
                eng.dma_start(xh_sb[j][:, c0:c1], xh[j][:, c0:c1])

            ub_sb = consts.tile([128, KT_X * R], dt_act, tag="ub")
            nc.sync.dma_start(ub_sb[:], ub[:])
            wc_sb = consts.tile([128, KT_C * R + KT_C * BS], dt_act, tag="wc")
            nc.scalar.dma_start(wc_sb[:], wc[:])
            Bc_sb = consts.tile([128, RT], dt_f32, tag="bc")
            nc.scalar.dma_start(Bc_sb[:], Bc2[:])
            load_x_chunk(nc.sync, 0, 0)
            load_x_chunk(nc.sync, 0, 1)
            vb_sb = consts.tile([128, RT * UNITS], dt_act, tag="vb")
            nc.scalar.dma_start(vb_sb[:], vb[:])
            load_x_chunk(nc.scalar, 1, 0)
            load_x_chunk(nc.sync, 1, 1)

            W_off = 0
            ctx_off = KT_C * R

            # ---- stage 1: h.T, chi.T (all b at once) ----
            chi_sb = consts.tile([128, RT * BS], dt_f32, tag="chi")
            for rh in range(RT):
                ps = ps_h.tile([128, BS], dt_f32, tag="hps")
                for n in range(KT_C):
                    nc.tensor.matmul(
                        ps[:],
                        wc_sb[:, W_off + n * R + rh * 128:
                                 W_off + n * R + rh * 128 + 128],
                        wc_sb[:, ctx_off + n * BS: ctx_off + (n + 1) * BS],
                        start=(n == 0), stop=(n == KT_C - 1))
                nc.scalar.activation(
                    chi_sb[:, rh * BS:(rh + 1) * BS], ps[:],
                    mybir.ActivationFunctionType.Sigmoid,
                    bias=Bc_sb[:, rh:rh + 1])

            # keep PE warm across the x-load gap
            warm_ps2 = ps_p.tile([128, 128], dt_f32, tag="pps")
            for _ in range(24):
                nc.tensor.matmul(warm_ps2[:], warm_sb[:], warm_sb[:],
                                 start=True, stop=True)
            nc.vector.tensor_copy(warm_sink[:, 16:], warm_ps2[:, :8])

            # ---- per b-slice: proj.T -> psT ; finals lag one slice ----
            psT_sb = consts.tile([128, RT * BS], dt_act, tag="psT")

            def emit_proj(j):
                for rh in range(RT):
                    ps = ps_p.tile([128, bh], dt_f32, tag="pps")
                    for k in range(KT_X):
                        nc.tensor.matmul(
                            ps[:],
                            ub_sb[:, k * R + rh * 128: k * R + rh * 128 + 128],
                            xh_sb[j][:, (k // kpc) * kpc * bh
                                     + (k % kpc) * bh:
                                     (k // kpc) * kpc * bh
                                     + (k % kpc) * bh + bh],
                            start=(k == 0), stop=(k == KT_X - 1))
                    nc.vector.tensor_mul(
                        psT_sb[:, rh * BS + j * bh: rh * BS + (j + 1) * bh],
                        ps[:],
                        chi_sb[:, rh * BS + j * bh: rh * BS + (j + 1) * bh])

            def emit_final(j):
                for t in range(bt_per_h):
                    bt = j * bt_per_h + t
                    o_sb = osb.tile([128, UNITS], dt_out, tag="o_sb")
                    for q in range(4):
                        ps = ps_o.tile([128, 512], dt_f32, tag="ops")
                        for rh in range(RT):
                            nc.tensor.matmul(
                                ps[:],
                                psT_sb[:, rh * BS + bt * 128:
                                          rh * BS + bt * 128 + 128],
                                vb_sb[:, rh * UNITS + q * 512:
                                      rh * UNITS + q * 512 + 512],
                                start=(rh == 0), stop=(rh == RT - 1))
                        dst = o_sb[:, q * 512:(q + 1) * 512]
                        if q % 2:
                            nc.scalar.activation(
                                dst, ps[:],
                                mybir.ActivationFunctionType.Copy)
                        else:
                            nc.vector.tensor_copy(dst, ps[:])
                        if q == 1:
                            nc.sync.dma_start(
                                out[bt * 128:(bt + 1) * 128, :UNITS // 2],
                                o_sb[:, :UNITS // 2])
                    nc.sync.dma_start(
                        out[bt * 128:(bt + 1) * 128, UNITS // 2:],
                        o_sb[:, UNITS // 2:])

            # software pipeline: each b-slice's finals follow its proj;
            # finals of slice j overlap the x loads of slice j+1
            for j in range(nh):
                emit_proj(j)
                emit_final(j)

            nc.sync.dma_start(dummy_out[:], warm_sink[:])

    nc.compile()
    return nc


def _get_nc(key):
    if key not in _COMPILED:
        _COMPILED[key] = _build(key)
    return _COMPILED[key]


def _pack(a, p=128):
    """[n*p, m] row-major -> [p, n*m]: partition p holds rows p, p+128, ..."""
    n = a.shape[0] // p
    return np.ascontiguousarray(
        a.reshape(n, p, a.shape[1]).transpose(1, 0, 2).reshape(p, -1))


def _prep_in_maps(inputs, context, U, S, V, W, Bc, act_dtype, nh):
    np_act = ml_dtypes.bfloat16 if act_dtype == "bf16" else np.float32
    bh = BS // nh

    Us = np.asarray(U, np.float32) * np.asarray(S, np.float32)[None, :]
    ub = _pack(Us).astype(np_act)
    vb = _pack(np.ascontiguousarray(np.asarray(V, np.float32).T)).astype(np_act)
    W32 = np.asarray(W, np.float32)
    Bc2 = np.ascontiguousarray(
        np.asarray(Bc, np.float32).reshape(RT, 128).T)

    x = np.asarray(inputs, np.float32)
    ctx = np.asarray(context, np.float32)
    in_maps = []
    for c in range(N_CORES):
        ctxT = ctx[c * BS:(c + 1) * BS, :].T
        wcb = np.concatenate([_pack(W32), _pack(np.ascontiguousarray(ctxT))],
                             axis=1).astype(np_act)
        xT = x[c * BS:(c + 1) * BS, :].T
        m = {"wc": wcb, "ub": ub, "vb": vb, "Bc2": Bc2}
        for j in range(nh):
            m[f"xh{j}"] = _pack(
                np.ascontiguousarray(xT[:, j * bh:(j + 1) * bh])).astype(np_act)
        in_maps.append(m)
    return in_maps


def kernel(inputs, context, U, S, V, W, Bc, bias, _run_kwargs=None):
    key = (ACT_DTYPE, OUT_BF16, NH, XCH, N_WARM)
    nc = _get_nc(key)
    in_maps = _prep_in_maps(inputs, context, U, S, V, W, Bc, ACT_DTYPE, NH)
    res = run_bass_kernel_spmd(nc, in_maps, list(range(N_CORES)),
                               **(_run_kwargs or {}))
    if _run_kwargs:
        kernel.last_results = res
    out = np.concatenate([np.asarray(res.results[c]["out"]).astype(np.float32)
                          for c in range(N_CORES)], axis=0)
    out += np.asarray(bias, np.float32)[None, :]
    return out


# revision 25
# speedup vs baseline: 1.0838x; 1.0838x over previous
"""Trainium2 Bass kernel for nn_CADenseMul.

Math (see reference):
    chi  = sigmoid(context @ W + Bc)          # [B, R]
    s    = S * chi                            # [B, R]
    out  = ((inputs @ U) * s) @ V.T + bias    # [B, UNITS]

Strategy:
  - Data-parallel over batch B across 8 cores (B=4096 -> 512 rows/core).
  - Host-side prep (not device time): per-core transposed activation shards
    packed into SBUF-layout blobs ([128, cols] contiguous per partition ->
    line-rate DMA); fold S into U (U_s = U * S); ship V pre-transposed;
    cast streams to bf16.
  - Device (transposed-activation layout, batch as the free dim):
        h.T    = W.T @ ctx.T          (PSUM; sigmoid+Bc on ACT)
        proj.T = U_s.T @ x.T          (per b-slice, pipelined with x loads)
        psT    = proj.T * chi.T       (DVE, cast bf16)
        out    = psT.T @ V.T          (psT stationary, natural-layout out)
  - PE warm-up: dummy matmuls at start so HAM un-throttles before real work.
  - Output stored bf16 (halves store traffic); host concats, adds bias fp32.
"""

import os
import numpy as np
import ml_dtypes

import concourse.bass as bass
import concourse.tile as tile
from concourse import bacc, mybir
from concourse.bass_utils import run_bass_kernel_spmd

N_CORES = 8
B, D_IN, D_CTX, UNITS, R = 4096, 2048, 512, 2048, 256
BS = B // N_CORES        # 512 batch rows per core
KT_X = D_IN // 128       # 16
KT_C = D_CTX // 128      # 4
RT = R // 128            # 2
NBT = BS // 128          # 4 output batch tiles

ACT_DTYPE = os.environ.get("CAD_DTYPE", "bf16")    # bf16 | f32r
OUT_BF16 = os.environ.get("CAD_OUT", "bf16") == "bf16"
# batch slice widths (multiples of 128 summing to BS) and x-DMA chunks each
SLICES = tuple(int(s) for s in
               os.environ.get("CAD_SLICES", "256,128,128").split(","))
XCHS = tuple(int(s) for s in
             os.environ.get("CAD_XCHS", "2,1,1").split(","))
N_WARM = int(os.environ.get("CAD_WARM", "60"))     # warm-up matmuls
N_WARM2 = int(os.environ.get("CAD_WARM2", "24"))   # gap-filler matmuls
assert sum(SLICES) == BS and len(XCHS) == len(SLICES)

_COMPILED = {}


def _build(key):
    act_dtype, out_bf16, slices, xchs, n_warm, n_warm2 = key
    dt_act = mybir.dt.bfloat16 if act_dtype == "bf16" else mybir.dt.float32r
    dt_f32 = mybir.dt.float32
    dt_out = mybir.dt.bfloat16 if out_bf16 else dt_f32
    nh = len(slices)
    boff = [sum(slices[:j]) for j in range(nh)]

    nc = bacc.Bacc("TRN2", target_bir_lowering=False, debug=False,
                   num_devices=N_CORES)

    # packed blobs: [128, cols] per-partition-contiguous
    wc = nc.dram_tensor("wc", [128, KT_C * R + KT_C * BS], dt_act,
                        kind="ExternalInput").ap()          # W | ctxT
    ub = nc.dram_tensor("ub", [128, KT_X * R], dt_act,
                        kind="ExternalInput").ap()          # U_s
    xh = [nc.dram_tensor(f"xh{j}", [128, KT_X * slices[j]], dt_act,
                         kind="ExternalInput").ap() for j in range(nh)]
    vb = nc.dram_tensor("vb", [128, RT * UNITS], dt_act,
                        kind="ExternalInput").ap()          # V.T
    Bc2 = nc.dram_tensor("Bc2", [128, RT], dt_f32, kind="ExternalInput").ap()
    out = nc.dram_tensor("out", [BS, UNITS], dt_out, kind="ExternalOutput").ap()
    dummy_out = nc.dram_tensor("dummy_out", [128, 24], dt_f32,
                               kind="ExternalOutput").ap()

    with tile.TileContext(nc) as tc:
        with (
            tc.tile_pool(name="consts", bufs=1) as consts,
            tc.tile_pool(name="osb", bufs=2) as osb,
            tc.tile_pool(name="ps_h", bufs=RT, space="PSUM") as ps_h,
            tc.tile_pool(name="ps_p", bufs=2, space="PSUM") as ps_p,
            tc.tile_pool(name="ps_o", bufs=4, space="PSUM") as ps_o,
        ):
            # ---- PE warm-up: garbage matmuls, no data deps ----
            warm_sb = consts.tile([128, 128], dt_act, tag="warm")
            nc.gpsimd.memset(warm_sb[:], 0.0)
            warm_ps = ps_p.tile([128, 128], dt_f32, tag="pps")
            for _ in range(n_warm):
                nc.tensor.matmul(warm_ps[:], warm_sb[:], warm_sb[:],
                                 start=True, stop=True)
            # keep it alive through DCE: route result to a real output
            warm_sink = consts.tile([128, 24], dt_f32, tag="warm_sink")
            nc.vector.tensor_copy(warm_sink[:, :8], warm_ps[:, :8])
            # preload the ACT "Copy" function table while ACT is idle so the
            # final-stage copies don't pay the 1.3us table load mid-kernel
            # (reads warm_sb, NOT warm_ps -- must not wait on the matmuls)
            nc.scalar.activation(warm_sink[:, 8:16], warm_sb[:, :8],
                                 mybir.ActivationFunctionType.Copy)

            # ---- loads: <=8 in flight (HWDGE sem lanes), split so arrival
            # order matches consumption: ub+x0 / wc early, vb mid, x-tail last
            xh_sb = []
            for j in range(nh):
                xt = consts.tile([128, KT_X * slices[j]], dt_act, tag=f"xh{j}")
                xh_sb.append(xt)

            def load_x_chunk(eng, j, q):
                kpc = KT_X // xchs[j]
                c0, c1 = q * kpc * slices[j], (q + 1) * kpc * slices[j]
                eng.dma_start(xh_sb[j][:, c0:c1], xh[j][:, c0:c1])

            ub_sb = consts.tile([128, KT_X * R], dt_act, tag="ub")
            nc.sync.dma_start(ub_sb[:], ub[:])
            wc_sb = consts.tile([128, KT_C * R + KT_C * BS], dt_act, tag="wc")
            nc.scalar.dma_start(wc_sb[:], wc[:])
            Bc_sb = consts.tile([128, RT], dt_f32, tag="bc")
            nc.scalar.dma_start(Bc_sb[:], Bc2[:])
            for q in range(xchs[0]):
                load_x_chunk(nc.sync, 0, q)
            vb_sb = consts.tile([128, RT * UNITS], dt_act, tag="vb")
            nc.scalar.dma_start(vb_sb[:], vb[:])
            for j in range(1, nh):
                for q in range(xchs[j]):
                    load_x_chunk(nc.scalar if j == nh - 1 else nc.sync, j, q)

            W_off = 0
            ctx_off = KT_C * R

            # ---- stage 1: h.T, chi.T (all b at once) ----
            chi_sb = consts.tile([128, RT * BS], dt_f32, tag="chi")
            for rh in range(RT):
                ps = ps_h.tile([128, BS], dt_f32, tag="hps")
                for n in range(KT_C):
                    nc.tensor.matmul(
                        ps[:],
                        wc_sb[:, W_off + n * R + rh * 128:
                                 W_off + n * R + rh * 128 + 128],
                        wc_sb[:, ctx_off + n * BS: ctx_off + (n + 1) * BS],
                        start=(n == 0), stop=(n == KT_C - 1))
                nc.scalar.activation(
                    chi_sb[:, rh * BS:(rh + 1) * BS], ps[:],
                    mybir.ActivationFunctionType.Sigmoid,
                    bias=Bc_sb[:, rh:rh + 1])

            # keep PE warm across the x-load gap
            warm_ps2 = ps_p.tile([128, 128], dt_f32, tag="pps")
            for _ in range(n_warm2):
                nc.tensor.matmul(warm_ps2[:], warm_sb[:], warm_sb[:],
                                 start=True, stop=True)
            nc.vector.tensor_copy(warm_sink[:, 16:], warm_ps2[:, :8])

            # ---- per b-slice: proj.T -> psT ; finals lag one slice ----
            psT_sb = consts.tile([128, RT * BS], dt_act, tag="psT")

            def emit_proj(j):
                bw = slices[j]
                for rh in range(RT):
                    ps = ps_p.tile([128, bw], dt_f32, tag="pps")
                    for k in range(KT_X):
                        nc.tensor.matmul(
                            ps[:],
                            ub_sb[:, k * R + rh * 128: k * R + rh * 128 + 128],
                            xh_sb[j][:, k * bw: (k + 1) * bw],
                            start=(k == 0), stop=(k == KT_X - 1))
                    nc.vector.tensor_mul(
                        psT_sb[:, rh * BS + boff[j]:
                                  rh * BS + boff[j] + bw],
                        ps[:],
                        chi_sb[:, rh * BS + boff[j]:
                                  rh * BS + boff[j] + bw])

            def emit_final(j):
                for t in range(slices[j] // 128):
                    bt = boff[j] // 128 + t
                    o_sb = osb.tile([128, UNITS], dt_out, tag="o_sb")
                    for q in range(4):
                        ps = ps_o.tile([128, 512], dt_f32, tag="ops")
                        for rh in range(RT):
                            nc.tensor.matmul(
                                ps[:],
                                psT_sb[:, rh * BS + bt * 128:
                                          rh * BS + bt * 128 + 128],
                                vb_sb[:, rh * UNITS + q * 512:
                                      rh * UNITS + q * 512 + 512],
                                start=(rh == 0), stop=(rh == RT - 1))
                        dst = o_sb[:, q * 512:(q + 1) * 512]
                        if q % 2:
                            nc.scalar.activation(
                                dst, ps[:],
                                mybir.ActivationFunctionType.Copy)
                        else:
                            nc.vector.tensor_copy(dst, ps[:])
                        if q == 1:
                            nc.sync.dma_start(
                                out[bt * 128:(bt + 1) * 128, :UNITS // 2],
                                o_sb[:, :UNITS // 2])
                    nc.sync.dma_start(
                        out[bt * 128:(bt + 1) * 128, UNITS // 2:],
                        o_sb[:, UNITS // 2:])

            # software pipeline: each b-slice's finals follow its proj;
            # finals of slice j overlap the x loads of slice j+1
            for j in range(nh):
                emit_proj(j)
                emit_final(j)

            nc.sync.dma_start(dummy_out[:], warm_sink[:])

    nc.compile()
    return nc


def _get_nc(key):
    if key not in _COMPILED:
        _COMPILED[key] = _build(key)
    return _COMPILED[key]


def _pack(a, p=128):
    """[n*p, m] row-major -> [p, n*m]: partition p holds rows p, p+128, ..."""
    n = a.shape[0] // p
    return np.ascontiguousarray(
        a.reshape(n, p, a.shape[1]).transpose(1, 0, 2).reshape(p, -1))


def _prep_in_maps(inputs, context, U, S, V, W, Bc, act_dtype, slices):
    np_act = ml_dtypes.bfloat16 if act_dtype == "bf16" else np.float32
    boff = [sum(slices[:j]) for j in range(len(slices))]

    Us = np.asarray(U, np.float32) * np.asarray(S, np.float32)[None, :]
    ub = _pack(Us).astype(np_act)
    vb = _pack(np.ascontiguousarray(np.asarray(V, np.float32).T)).astype(np_act)
    W32 = np.asarray(W, np.float32)
    Bc2 = np.ascontiguousarray(
        np.asarray(Bc, np.float32).reshape(RT, 128).T)

    x = np.asarray(inputs, np.float32)
    ctx = np.asarray(context, np.float32)
    in_maps = []
    for c in range(N_CORES):
        ctxT = ctx[c * BS:(c + 1) * BS, :].T
        wcb = np.concatenate([_pack(W32), _pack(np.ascontiguousarray(ctxT))],
                             axis=1).astype(np_act)
        xT = x[c * BS:(c + 1) * BS, :].T
        m = {"wc": wcb, "ub": ub, "vb": vb, "Bc2": Bc2}
        for j, bw in enumerate(slices):
            m[f"xh{j}"] = _pack(np.ascontiguousarray(
                xT[:, boff[j]:boff[j] + bw])).astype(np_act)
        in_maps.append(m)
    return in_maps


def kernel(inputs, context, U, S, V, W, Bc, bias, _run_kwargs=None):
    key = (ACT_DTYPE, OUT_BF16, SLICES, XCHS, N_WARM, N_WARM2)
    nc = _get_nc(key)
    in_maps = _prep_in_maps(inputs, context, U, S, V, W, Bc, ACT_DTYPE, SLICES)
    res = run_bass_kernel_spmd(nc, in_maps, list(range(N_CORES)),
                               **(_run_kwargs or {}))
    if _run_kwargs:
        kernel.last_results = res
    out = np.concatenate([np.asarray(res.results[c]["out"]).astype(np.float32)
                          for c in range(N_CORES)], axis=0)
    out += np.asarray(bias, np.float32)[None, :]
    return out
